# revision 15
# baseline (speedup 1.0000x reference)
"""DGCNN (3x dynamic EdgeConv + MLP head) on 8 Trainium2 NeuronCores."""


import numpy as np

import bass_rust
import concourse.bass as bass
import concourse.mybir as mybir
import concourse.tile as tile
from concourse import bacc, library_config
from concourse.bass_types import AP
from concourse.bass import IndirectOffsetOnAxis

F32 = mybir.dt.float32
F32R = mybir.dt.float32r
F16 = mybir.dt.float16
U32 = mybir.dt.uint32
I16 = mybir.dt.int16
AF = mybir.ActivationFunctionType
ALU = mybir.AluOpType
AX = mybir.AxisListType

NEG_BIG = -3.0e38


def _r(ap):
    return ap  # fp32 matmuls for now; fp32r needs a rounding producer chain


def split_excess_waits(nc, maxw=1):
    """This walrus build only encodes one sem-wait on some CTRL opcodes;
    redistribute excess waits onto EventSemaphore carrier instructions."""
    nid = 0
    for f in nc.m.functions:
        for bb in f.blocks:
            new_insts = []
            for inst in bb.instructions:
                si = inst.sync_info
                if si is not None and si.on_wait and len(si.on_wait) > maxw:
                    waits = list(si.on_wait)
                    excess, keep = waits[:-maxw], waits[-maxw:]
                    for w in excess:
                        nid += 1
                        carrier = mybir.InstEventSemaphore(
                            name=f"WSPLIT-{nid}", ins=[], outs=[])
                        carrier.engine = inst.engine
                        carrier.sync_info = bass_rust.SyncInfo(
                            on_wait=[w], on_update=[])
                        new_insts.append(carrier)
                    si.on_wait = keep
                new_insts.append(inst)
            bb.instructions[:] = new_insts


# Gather output on every core and fetch one shard vs fetch 8 shards
# concurrently: measured identical (fetch is RTT-bound, ~100ms either
# way); multi-shard avoids the cross-core barrier, so keep it off.
ALLGATHER = False


def build_program(P=2048, G=2, K=30, KP=32):
    """Returns the Bass program. P: points per graph, G: graphs per core."""
    NT = P // 128            # itiles per graph
    GRP = min(4, NT)         # itiles per gather group
    NS = min(512, P)         # matmul N-chunk
    NN = P // NS
    NGRP = NT // GRP
    NI_G = GRP * 128 * KP    # num_idxs per gather group

    nc = bacc.Bacc("TRN2", target_bir_lowering=False, debug=False,
                   num_devices=8, num_swdge_queues=4)

    def param(name, shape, dtype):
        return nc.declare_dram_parameter(name, list(shape), dtype,
                                         isOutput=False)

    xt1 = param("xt1", [G, 4, P], F32)   # [xT; ones] per graph (host-built)
    xr1 = param("xr1", [G, 4, P], F32)   # [2xT; -sq] per graph (host-built)
    w_u, w_v, w_b16, b_b = [], [], [], []
    for l, Fin in ((1, 3), (2, 64), (3, 64)):
        w_u.append(param(f"w{l}u", [Fin + 1, 64], F32))  # bias row appended
        w_v.append(param(f"w{l}v", [Fin, 64], F32))
        w_b16.append(param(f"w{l}b", [64, 64], F16))
        b_b.append(param(f"b{l}b", [64], F32))
    wla_d = param("wla", [128, 1024], F32)
    wlb_d = param("wlb", [64, 1024], F32)
    bl_d = param("bl", [1024], F32)
    wm1_d = param("wm1", [8, 128, 256], F32)
    bm1_d = param("bm1", [256], F32)
    wm2_d = param("wm2", [2, 128, 128], F32)
    bm2_d = param("bm2", [128], F32)
    wout_d = param("wout", [128, 7], F32)
    bout_d = param("bout", [7], F32)
    ident_d = param("ident", [128, 128], F32)

    # after an in-kernel AllGather every core holds the full [B*P, 7]
    # output, so the host fetches a single shard (one tunnel stream)
    out_rows = 8 * G * P if ALLGATHER else G * P
    out_d = nc.declare_dram_parameter("out", [out_rows, 7], F16,
                                      isOutput=True)

    with tile.TileContext(nc) as tc:
        consts = tc.alloc_tile_pool(name="consts", bufs=1)
        per_g = tc.alloc_tile_pool(name="per_g", bufs=1)
        per_l = tc.alloc_tile_pool(name="per_l", bufs=1)
        stream = tc.alloc_tile_pool(name="stream", bufs=2)
        dramp = tc.alloc_tile_pool(name="dramp", bufs=2, space="DRAM")
        dramo = tc.alloc_tile_pool(name="dramo", bufs=1, space="DRAM")
        out_local = dramo.tile([G * P, 7], F16, tag="out_local")
        out_all = dramo.tile([8 * G * P, 7], F16, tag="out_all")
        psA = tc.alloc_tile_pool(name="psA", bufs=1, space="PSUM")
        psB = tc.alloc_tile_pool(name="psB", bufs=2, space="PSUM")
        psT = tc.alloc_tile_pool(name="psT", bufs=2, space="PSUM")

        # ---------------- constants ----------------
        wu_sb, wv_sb, wb_sb, bb_sb = [], [], [], []
        for l in range(3):
            Fin = 3 if l == 0 else 64
            t = consts.tile([Fin + 1, 64], F32, name=f"wu{l}", tag=f"wu{l}")
            nc.sync.dma_start(out=t, in_=w_u[l][:])
            wu_sb.append(t)
            t = consts.tile([Fin, 64], F32, name=f"wv{l}", tag=f"wv{l}")
            nc.sync.dma_start(out=t, in_=w_v[l][:])
            wv_sb.append(t)
            t = consts.tile([64, 64], F16, name=f"wb{l}", tag=f"wb{l}")
            nc.sync.dma_start(out=t, in_=w_b16[l][:])
            wb_sb.append(t)
            t = consts.tile([64, 1], F32, name=f"bb{l}", tag=f"bb{l}")
            nc.sync.dma_start(out=t, in_=AP(b_b[l], 0, [[1, 64], [0, 1]]))
            bb_sb.append(t)
        wla = consts.tile([128, 1024], F32, tag="wla")
        nc.sync.dma_start(out=wla, in_=wla_d[:])
        wlb = consts.tile([64, 1024], F32, tag="wlb")
        nc.sync.dma_start(out=wlb, in_=wlb_d[:])
        bl_sb = consts.tile([128, 8], F32, tag="bl")
        nc.sync.dma_start(out=bl_sb, in_=AP(bl_d, 0, [[1, 128], [128, 8]]))
        wm1 = consts.tile([128, 8, 256], F32, tag="wm1")
        nc.sync.dma_start(
            out=wm1,
            in_=AP(wm1_d, 0, [[256, 128], [128 * 256, 8], [1, 256]]))
        bm1_sb = consts.tile([128, 2], F32, tag="bm1")
        nc.sync.dma_start(out=bm1_sb, in_=AP(bm1_d, 0, [[1, 128], [128, 2]]))
        wm2 = consts.tile([128, 2, 128], F32, tag="wm2")
        nc.sync.dma_start(
            out=wm2,
            in_=AP(wm2_d, 0, [[128, 128], [128 * 128, 2], [1, 128]]))
        bm2_sb = consts.tile([128, 1], F32, tag="bm2")
        nc.sync.dma_start(out=bm2_sb, in_=AP(bm2_d, 0, [[1, 128], [0, 1]]))
        wout = consts.tile([128, 7], F32, tag="wout")
        nc.sync.dma_start(out=wout, in_=wout_d[:])
        bout_sb = consts.tile([7, 1], F32, tag="bout")
        nc.sync.dma_start(out=bout_sb, in_=AP(bout_d, 0, [[1, 7], [0, 1]]))
        ident = consts.tile([128, 128], F32, tag="ident")
        nc.sync.dma_start(out=ident, in_=ident_d[:])

        x123s = []

        def emit_mlp_chunk(gm, ntc):
            """One NS-row chunk of the final MLP for graph gm. Emitted out
            of line so graph 0's chunks can be woven into graph 1's layer-0
            itile loop: they run on PE/Act/DVE slack under graph 1's gather
            stream instead of stalling Pool for ~200us between graphs."""
            x123a, x123b = x123s[gm]
            csl = slice(ntc * NS, (ntc + 1) * NS)
            hl = per_l.tile([128, 8, NS], F32, name="hl", tag="hl")
            for m in range(8):
                pl = psB.tile([128, NS], F32, name="pl", tag="small")
                nc.tensor.matmul(pl, _r(wla[:, m * 128:(m + 1) * 128]),
                                 _r(x123a[:, csl]), start=True, stop=False)
                nc.tensor.matmul(pl, _r(wlb[:, m * 128:(m + 1) * 128]),
                                 _r(x123b[:, csl]), start=False, stop=True)
                nc.scalar.activation(hl[:, m, :], pl, AF.Relu,
                                     bias=bl_sb[:, m:m + 1])
            hm1 = per_l.tile([128, 2, NS], F32, name="hm1", tag="hm1")
            for m in range(2):
                pm = psB.tile([128, NS], F32, name="pm", tag="small")
                for kk in range(8):
                    nc.tensor.matmul(pm, _r(wm1[:, kk, m * 128:(m + 1) * 128]),
                                     _r(hl[:, kk, :]), start=(kk == 0),
                                     stop=(kk == 7))
                nc.scalar.activation(hm1[:, m, :], pm, AF.Relu,
                                     bias=bm1_sb[:, m:m + 1])
            hm2 = per_l.tile([128, NS], F32, name="hm2", tag="hm2")
            pm2 = psB.tile([128, NS], F32, name="pm2", tag="small")
            for kk in range(2):
                nc.tensor.matmul(pm2, _r(wm2[:, kk, :]), _r(hm1[:, kk, :]),
                                 start=(kk == 0), stop=(kk == 1))
            nc.scalar.activation(hm2, pm2, AF.Relu, bias=bm2_sb)
            po = psB.tile([7, NS], F32, name="po", tag="small",
                          padded_shape=[128, 512])
            nc.tensor.matmul(po, _r(wout), _r(hm2), start=True, stop=True)
            oT = per_l.tile([7, NS], F32, name="oT", tag="oT")
            nc.vector.tensor_scalar_add(oT, po, bout_sb)
            for s in range(NS // 128):
                pt = psB.tile([128, 7], F32, name="pt", tag="small")
                nc.tensor.transpose(
                    pt, oT[:, s * 128:(s + 1) * 128], ident[0:7, 0:7])
                ot = per_l.tile([128, 7], F32, name="ot", tag="ot")
                nc.scalar.activation(ot, pt, AF.Copy)
                nsq = per_l.tile([128, 7], F32, name="nsq", tag="nsq")
                nc.vector.tensor_tensor(out=nsq, in0=ot, in1=ot,
                                        op=ALU.mult)
                nrm = per_l.tile([128, 1], F32, name="nrm", tag="nrm")
                nc.vector.tensor_reduce(out=nrm, in_=nsq, axis=AX.X,
                                        op=ALU.add)
                nc.scalar.activation(nrm, nrm, AF.Sqrt)
                nc.vector.tensor_scalar_max(nrm, nrm, 1e-12)
                nc.vector.reciprocal(nrm, nrm)
                ot16 = per_l.tile([128, 7], F16, name="ot16", tag="ot16")
                nc.vector.tensor_scalar_mul(ot16, ot, nrm)
                row0 = gm * P + ntc * NS + s * 128
                dst_buf = out_local if ALLGATHER else out_d
                nc.sync.dma_start(out=dst_buf[row0:row0 + 128, :],
                                  in_=ot16)

        for g in range(G):
            x123a = per_g.tile([128, P], F32, name=f"x123a_g{g}",
                               tag=f"x123a_g{g}")
            x123b = per_g.tile([64, P], F32, name=f"x123b_g{g}",
                               tag=f"x123b_g{g}")
            x123s.append((x123a, x123b))

            for l in range(3):
                Fin = 3 if l == 0 else 64

                xt = per_l.tile([Fin + 1, P], F32, name="xt", tag="xt",
                                padded_shape=[65, P])
                xr = per_l.tile([Fin + 1, P], F32, name="xr", tag="xr",
                                padded_shape=[65, P])
                if l == 0:
                    nc.sync.dma_start(out=xt, in_=xt1[g])
                    nc.sync.dma_start(out=xr, in_=xr1[g])
                else:
                    src = x123a[0:64, :] if l == 1 else x123a[64:128, :]
                    nc.scalar.activation(xt[0:64, :], src, AF.Copy)
                    nc.vector.memset(xt[64:65, :], 1.0)
                    nc.scalar.activation(xr[0:64, :], src, AF.Copy, scale=2.0)
                    xsq = per_l.tile([64, P], F32, name="xsq", tag="xsq")
                    nc.vector.tensor_tensor(out=xsq, in0=src, in1=src,
                                            op=ALU.mult)
                    onesl = per_l.tile([64, 1], F32, name="onesl", tag="onesl")
                    nc.vector.memset(onesl, 1.0)
                    for n in range(NN):
                        sqp = psB.tile([1, NS], F32, name="sqp", tag="small",
                                       padded_shape=[1, 512])
                        nc.tensor.matmul(sqp, _r(onesl),
                                         _r(xsq[:, n * NS:(n + 1) * NS]),
                                         start=True, stop=True)
                        nc.scalar.activation(xr[64:65, n * NS:(n + 1) * NS],
                                             sqp, AF.Copy, scale=-1.0)

                def pm_transpose(src_t, dst_t):
                    # feature-major [64, P] -> point-major [128, NT, 64]
                    for c in range((NT + 7) // 8):
                        s0 = c * 8
                        ns = min(8, NT - s0)
                        tpb = psT.tile([128, 8 * 64], F32, name="tpb", tag="tpb")
                        for s in range(s0, s0 + ns):
                            nc.tensor.transpose(
                                tpb[:, (s - s0) * 64:(s - s0 + 1) * 64],
                                src_t[:, s * 128:(s + 1) * 128],
                                ident[0:64, 0:64])
                        nc.scalar.activation(
                            dst_t[:, s0:s0 + ns, :].rearrange("p a b -> p (a b)"),
                            tpb[:, 0:ns * 64], AF.Copy)

                # ---- v first: its DRAM copy gates the first gather of the
                # layer. The u-chain is only needed at the z+u add (well
                # after the first gather), so it is woven into the itile
                # loop at it==0, where it fills PE/Act slack during it0's
                # top-k instead of delaying the layer head.
                vp = psA.tile([64, P], F32, name="vp", tag="bigps")
                for n in range(NN):
                    nc.tensor.matmul(vp[:, n * NS:(n + 1) * NS], _r(wv_sb[l]),
                                     _r(xt[0:Fin, n * NS:(n + 1) * NS]),
                                     start=True, stop=True)
                vT = per_l.tile([64, P], F32, name="vT", tag="vT")
                nc.scalar.activation(vT, vp, AF.Copy)
                vP = per_l.tile([128, NT, 64], F32, name="vP", tag="vP")
                pm_transpose(vT, vP)
                v_dram = dramp.tile([P, 64], F32, name="v_dram", tag="v_dram")
                nc.sync.dma_start(
                    out=v_dram.rearrange("(s p) f -> p s f", p=128), in_=vP)

                uP = per_l.tile([128, NT, 64], F32, name="uP", tag="uP")
                xrawT = per_l.tile([64, P], F32, name="xrawT", tag="xrawT")

                for it in range(NT):
                    # ---- scores (one itile vs all points) ----
                    scp = psA.tile([128, P], F32, name="scp", tag="bigps")
                    for n in range(NN):
                        nc.tensor.matmul(
                            scp[:, n * NS:(n + 1) * NS],
                            _r(xt[:, it * 128:(it + 1) * 128]),
                            _r(xr[:, n * NS:(n + 1) * NS]),
                            start=True, stop=True)
                    # scores stay f32: fp16 here saves ~1ms of DVE time
                    # but is RTT-masked in wall-clock and costs 10x
                    # accuracy margin (kNN flips on rounded distances)
                    sc = stream.tile([128, P], F32, name="sc", tag="sc",
                                     bufs=2)
                    nc.scalar.activation(sc, scp, AF.Copy)
                    if it == 0:
                        # u-chain (bias folded via ones row): runs on
                        # PE/Act while DVE does it0's top-k
                        up = psA.tile([64, P], F32, name="up", tag="bigps")
                        for n in range(NN):
                            nc.tensor.matmul(
                                up[:, n * NS:(n + 1) * NS], _r(wu_sb[l]),
                                _r(xt[:, n * NS:(n + 1) * NS]),
                                start=True, stop=True)
                        uT = per_l.tile([64, P], F32, name="uT", tag="uT16")
                        nc.scalar.activation(uT, up, AF.Copy)
                        pm_transpose(uT, uP)
                    # ---- top-30 (exact) ----
                    # A pair-max pre-reduction (select on max(sc[2q],
                    # sc[2q+1]) then value-match-recover indices) was tried
                    # and cut the DVE passes ~25%, but loses neighbors
                    # whenever both pair elements rank in the top-30 and on
                    # exact score ties (rel err 5e-3..1.6e-2, input-bit
                    # sensitive). The wall is gather-issue-bound, so the DVE
                    # saving bought no wall time — keep the exact scheme.
                    idxg = stream.tile([128, KP], U32, name="idxg", tag="idxg")
                    vals8 = stream.tile([128, 8], F32, name="vals8", tag="v8")
                    for rd in range(4):
                        nc.vector.max(out=vals8, in_=sc)
                        nc.vector.max_index(
                            out=idxg[:, rd * 8:(rd + 1) * 8],
                            in_max=vals8, in_values=sc)
                        if rd < 3:
                            nc.vector.match_replace(
                                out=sc, in_to_replace=vals8, in_values=sc,
                                imm_value=NEG_BIG)

                    # ---- gather neighbor features (point-major fp32) ----
                    # z is prefilled with u_i on Act; each per-k indirect
                    # gather (one offset per partition is all HW supports)
                    # adds v_j in the DMA compute engine, so h1-preact lands
                    # with zero DVE work. Only K=30 gathers — the baseline's
                    # 2 padded duplicate gathers are dropped.
                    z = stream.tile([128, K, 64], F32, name="z", tag="z",
                                    bufs=4)
                    u_b = uP[:, it, :].rearrange(
                        "p (one f) -> p one f", one=1).to_broadcast(
                        [128, K, 64])
                    # k=0 is always self (d2=0 is the strict row minimum;
                    # feature-space ties at 0 imply identical vectors), and
                    # v_self is already in SBUF — copy it instead of paying
                    # the ~1.6us fixed cost of a 30th indirect gather.
                    nc.scalar.activation(z[:, 0, :], vP[:, it, :], AF.Copy)
                    for kk in range(1, K):
                        nc.gpsimd.indirect_dma_start(
                            out=z[:, kk, :], out_offset=None, in_=v_dram[:],
                            in_offset=IndirectOffsetOnAxis(
                                ap=idxg[:, kk:kk + 1], axis=0))
                    nc.vector.tensor_tensor(out=z, in0=z, in1=u_b, op=ALU.add)
                    # ---- transpose to feature-major fp16, h2, max over k --
                    parts = per_l.tile([64, 8, 128], F32, name="parts",
                                       tag="parts")
                    for T in range(8):
                        bc = 4 if T < 7 else K - 28  # 30 k's: 7x4 + 1x2
                        tpz = psT.tile([64, 512], F32, name="tpz", tag="tpb")
                        for b in range(bc):
                            g2 = T * 4 + b
                            nc.tensor.transpose(
                                tpz[:, b * 128:(b + 1) * 128],
                                z[:, g2, :], ident)
                        h1T = stream.tile([64, 512], F16, name="h1T",
                                          tag="h1T", bufs=6)
                        nc.scalar.activation(h1T[:, 0:bc * 128],
                                             tpz[:, 0:bc * 128], AF.Relu)
                        hp = psB.tile([64, 512], F32, name="hp", tag="small")
                        nc.tensor.matmul(hp[:, 0:bc * 128], wb_sb[l],
                                         h1T[:, 0:bc * 128],
                                         start=True, stop=True)
                        nc.vector.tensor_reduce(
                            out=parts[:, T, :],
                            in_=hp.rearrange("p (b i) -> p i b", b=4)[
                                :, :, 0:bc],
                            axis=AX.X, op=ALU.max)
                    nc.vector.tensor_reduce(
                        out=xrawT[:, it * 128:(it + 1) * 128],
                        in_=parts.rearrange("p a b -> p b a"),
                        axis=AX.X, op=ALU.max)

                    # weave the PREVIOUS graph's MLP chunks under this
                    # graph's gather stream (one chunk per 4 itiles)
                    if g > 0 and l == 0 and it % 4 == 3:
                        emit_mlp_chunk(g - 1, it // 4)
                    # last graph, last layer: finalize x123b per 4-itile
                    # slice and emit that MLP chunk immediately, so the
                    # final MLP runs under this layer's remaining gathers
                    # instead of as a ~200us Pool-idle tail.
                    if g == G - 1 and l == 2 and it % 4 == 3:
                        sl = slice((it - 3) * 128, (it + 1) * 128)
                        nc.scalar.activation(x123b[0:64, sl], xrawT[:, sl],
                                             AF.Relu, bias=bb_sb[l])
                        emit_mlp_chunk(g, it // 4)

                if not (g == G - 1 and l == 2):
                    dst = (x123a[0:64, :] if l == 0 else
                           x123a[64:128, :] if l == 1 else x123b[0:64, :])
                    nc.scalar.activation(dst, xrawT, AF.Relu, bias=bb_sb[l])

        if ALLGATHER:
            nc.gpsimd.collective_compute(
                "AllGather", ALU.bypass,
                replica_groups=[list(range(8))],
                ins=[out_local.opt()], outs=[out_all.opt()])
            nc.sync.dma_start(out=out_d[:], in_=out_all[:])

        for pool in (psT, psB, psA, stream, per_l, per_g, consts):
            pool.release()
        dramp.release()
        dramo.release()

    nc.finalize()
    return nc


def prep_inputs(inputs, P=2048, G=2, n_cores=8):
    """Host-side prep: fold edge-MLP weights, build per-core input maps."""
    pos = np.asarray(inputs["pos"], np.float32).reshape(-1, P, 3)
    B = pos.shape[0]
    assert B == G * n_cores

    def fold(wa, ba, Fin):
        wa = np.asarray(wa, np.float32)
        wu = wa[:Fin] - wa[Fin:]
        wub = np.vstack([wu, np.asarray(ba, np.float32)[None, :]])
        return wub.astype(np.float32), wa[Fin:].astype(np.float32)

    w1u, w1v = fold(inputs["w1a"], inputs["b1a"], 3)
    w2u, w2v = fold(inputs["w2a"], inputs["b2a"], 64)
    w3u, w3v = fold(inputs["w3a"], inputs["b3a"], 64)

    shared = {
        "w1u": w1u, "w1v": w1v,
        "w1b": np.asarray(inputs["w1b"], np.float16),
        "b1b": np.asarray(inputs["b1b"], np.float32),
        "w2u": w2u, "w2v": w2v,
        "w2b": np.asarray(inputs["w2b"], np.float16),
        "b2b": np.asarray(inputs["b2b"], np.float32),
        "w3u": w3u, "w3v": w3v,
        "w3b": np.asarray(inputs["w3b"], np.float16),
        "b3b": np.asarray(inputs["b3b"], np.float32),
        "wla": np.ascontiguousarray(np.asarray(inputs["wl"], np.float32)[:128]),
        "wlb": np.ascontiguousarray(np.asarray(inputs["wl"], np.float32)[128:]),
        "bl": np.asarray(inputs["bl"], np.float32),
        "wm1": np.ascontiguousarray(
            np.asarray(inputs["wm1"], np.float32).reshape(8, 128, 256)),
        "bm1": np.asarray(inputs["bm1"], np.float32),
        "wm2": np.ascontiguousarray(
            np.asarray(inputs["wm2"], np.float32).reshape(2, 128, 128)),
        "bm2": np.asarray(inputs["bm2"], np.float32),
        "wout": np.asarray(inputs["wout"], np.float32),
        "bout": np.asarray(inputs["bout"], np.float32),
        "ident": np.eye(128, dtype=np.float32),
    }

    in_maps = []
    for c in range(n_cores):
        xg = pos[c * G:(c + 1) * G]                      # [G, P, 3]
        xT = np.transpose(xg, (0, 2, 1))                 # [G, 3, P]
        ones = np.ones((G, 1, P), np.float32)
        sq = np.sum(xg * xg, axis=2)[:, None, :]         # [G, 1, P]
        m = dict(shared)
        m["xt1"] = np.ascontiguousarray(
            np.concatenate([xT, ones], axis=1).astype(np.float32))
        m["xr1"] = np.ascontiguousarray(
            np.concatenate([2.0 * xT, -sq], axis=1).astype(np.float32))
        in_maps.append(m)
    return in_maps


# ----------------------------------------------------------------------------
# Public entry point: takes FULL inputs (as in reference.setup_inputs()),
# shards B=16 graphs across 8 NeuronCores (2 graphs/core), runs the SPMD
# Bass kernel, returns the FULL [B*P, 7] output.
# ----------------------------------------------------------------------------

_CACHE = {}


class _Res:
    exec_time_ns = None


def _get_runner():
    """Build the Bass program and a cached jitted shard_map executor.

    run_bass_kernel_spmd re-creates the jit closure per call, so every
    call re-traces, re-lowers and re-runs the walrus/NEFF pipeline
    (~1.1s). Building the jitted callable once keeps warm calls on the
    fast path: concat inputs -> cached pjit -> fetch outputs.
    """
    if "runner" in _CACHE:
        return _CACHE["runner"]

    import jax
    from jax.experimental.shard_map import shard_map
    from jax.sharding import Mesh, PartitionSpec
    from concourse import bass2jax

    nc = build_program(P=2048, G=2)
    bass2jax.install_neuronx_cc_hook()

    n_cores = 8
    partition_name = (nc.partition_id_tensor.name
                      if nc.partition_id_tensor else None)
    in_names, out_names, out_avals, zero_outs = [], [], [], []
    for alloc in nc.m.functions[0].allocations:
        if not isinstance(alloc, mybir.MemoryLocationSet):
            continue
        name = alloc.memorylocations[0].name
        if alloc.kind == "ExternalInput":
            if name != partition_name:
                in_names.append(name)
        elif alloc.kind == "ExternalOutput":
            shape = tuple(alloc.tensor_shape)
            dtype = mybir.dt.np(alloc.dtype)
            out_names.append(name)
            out_avals.append(jax.core.ShapedArray(shape, dtype))
            zero_outs.append(np.zeros(shape, dtype))
    n_params = len(in_names)
    n_outs = len(out_names)
    all_in = list(in_names) + list(out_names)
    if partition_name is not None:
        all_in.append(partition_name)
    donate = tuple(range(n_params, n_params + n_outs))

    def _body(*args):
        operands = list(args)
        if partition_name is not None:
            operands.append(bass2jax.partition_id_tensor())
        outs = bass2jax._bass_exec_p.bind(
            *operands,
            out_avals=tuple(out_avals),
            in_names=tuple(all_in),
            out_names=tuple(out_names),
            lowering_input_output_aliases=(),
            sim_require_finite=True,
            sim_require_nnan=True,
            nc=nc,
        )
        return tuple(outs)

    devices = jax.devices()[:n_cores]
    mesh = Mesh(np.asarray(devices), ("core",))
    in_specs = (PartitionSpec("core"),) * (n_params + n_outs)
    out_specs = (PartitionSpec("core"),) * n_outs
    sharded = jax.jit(
        shard_map(_body, mesh=mesh, in_specs=in_specs,
                  out_specs=out_specs, check_rep=False),
        donate_argnums=donate, keep_unused=True)

    sharding = jax.sharding.NamedSharding(mesh, PartitionSpec("core"))
    _CACHE["runner"] = (sharded, in_names, zero_outs, sharding)
    return _CACHE["runner"]


def _prep_weights_np(inputs):
    """Host-side fold of the edge-MLP weights into the kernel layout."""
    def fold(wa, ba, Fin):
        wa = np.asarray(wa, np.float32)
        wu = wa[:Fin] - wa[Fin:]
        wub = np.vstack([wu, np.asarray(ba, np.float32)[None, :]])
        return wub.astype(np.float32), wa[Fin:].astype(np.float32)

    w1u, w1v = fold(inputs["w1a"], inputs["b1a"], 3)
    w2u, w2v = fold(inputs["w2a"], inputs["b2a"], 64)
    w3u, w3v = fold(inputs["w3a"], inputs["b3a"], 64)
    wl = np.asarray(inputs["wl"], np.float32)
    return {
        "w1u": w1u, "w1v": w1v,
        "w1b": np.asarray(inputs["w1b"], np.float16),
        "b1b": np.asarray(inputs["b1b"], np.float32),
        "w2u": w2u, "w2v": w2v,
        "w2b": np.asarray(inputs["w2b"], np.float16),
        "b2b": np.asarray(inputs["b2b"], np.float32),
        "w3u": w3u, "w3v": w3v,
        "w3b": np.asarray(inputs["w3b"], np.float16),
        "b3b": np.asarray(inputs["b3b"], np.float32),
        "wla": np.ascontiguousarray(wl[:128]),
        "wlb": np.ascontiguousarray(wl[128:]),
        "bl": np.asarray(inputs["bl"], np.float32),
        "wm1": np.ascontiguousarray(
            np.asarray(inputs["wm1"], np.float32).reshape(8, 128, 256)),
        "bm1": np.asarray(inputs["bm1"], np.float32),
        "wm2": np.ascontiguousarray(
            np.asarray(inputs["wm2"], np.float32).reshape(2, 128, 128)),
        "bm2": np.asarray(inputs["bm2"], np.float32),
        "wout": np.asarray(inputs["wout"], np.float32),
        "bout": np.asarray(inputs["bout"], np.float32),
        "ident": np.eye(128, dtype=np.float32),
    }


def run_sharded(inputs, trace=False):
    """Run with device-side caching; on runtime errors (e.g. the terminal
    evicted a cached device buffer) drop caches and retry from host."""
    for attempt in range(3):
        try:
            return _run_sharded_once(inputs, trace=trace)
        except Exception:
            if attempt == 2:
                raise
            for k in ("wfp", "wdev", "posfp", "xdev", "prev_out"):
                _CACHE.pop(k, None)


def _run_sharded_once(inputs, trace=False):
    if trace:  # profiling path: NTFF capture + exec_time_ns
        from concourse.bass_utils import run_bass_kernel_spmd
        if "nc" not in _CACHE:
            _CACHE["nc"] = build_program(P=2048, G=2)
        in_maps = prep_inputs(inputs, P=2048, G=2, n_cores=8)
        res = run_bass_kernel_spmd(_CACHE["nc"], in_maps, list(range(8)),
                                   trace=True)
        out = np.concatenate([res.results[i]["out"] for i in range(8)],
                             axis=0)
        return out.astype(np.float32), res
    sharded, in_names, zero_outs, sharding = _get_runner()

    # Optimistic dispatch: if device-resident inputs exist, launch with
    # them immediately and verify the input fingerprints while the RPC
    # is in flight (~4ms of hashing hidden under the ~100ms roundtrip).
    # On mismatch (inputs changed since the cache was built) the result
    # is discarded and the call re-runs with freshly uploaded inputs.
    if "xdev" in _CACHE and "wdev" in _CACHE:
        out_arrs = sharded(*_build_args(in_names), *_prev_bufs(
            zero_outs, sharding))
        posfp, wfp = _fingerprints(inputs)
        if posfp == _CACHE["posfp"] and wfp == _CACHE["wfp"]:
            return _fetch(out_arrs)
        _CACHE["prev_out"] = list(out_arrs)  # reuse as donation bufs
    else:
        posfp, wfp = _fingerprints(inputs)

    _refresh_device_inputs(inputs, posfp, wfp, sharding)
    out_arrs = sharded(*_build_args(in_names), *_prev_bufs(
        zero_outs, sharding))
    return _fetch(out_arrs)


def _fingerprints(inputs):
    import hashlib
    pos_raw = np.ascontiguousarray(np.asarray(inputs["pos"], np.float32))
    posfp = hashlib.blake2b(pos_raw, digest_size=16).digest()
    h = hashlib.blake2b(digest_size=16)
    for k in ("w1a", "b1a", "w1b", "b1b", "w2a", "b2a", "w2b", "b2b",
              "w3a", "b3a", "w3b", "b3b", "wl", "bl", "wm1", "bm1",
              "wm2", "bm2", "wout", "bout"):
        h.update(np.ascontiguousarray(inputs[k]))
    return posfp, h.digest()


def _refresh_device_inputs(inputs, posfp, wfp, sharding):
    import jax
    n_cores = 8
    if _CACHE.get("posfp") != posfp or "xdev" not in _CACHE:
        # pos-dependent tensors, built full-batch: concat over cores of
        # the per-core [G,4,P] params is just the [B,4,P] batch tensor
        pos = np.ascontiguousarray(
            np.asarray(inputs["pos"], np.float32)).reshape(16, 2048, 3)
        xT = np.transpose(pos, (0, 2, 1))
        ones = np.ones((16, 1, 2048), np.float32)
        sq = np.einsum('bpf,bpf->bp', pos, pos)[:, None, :]
        xt1 = np.ascontiguousarray(np.concatenate([xT, ones], axis=1))
        xr1 = np.ascontiguousarray(
            np.concatenate([2.0 * xT, -sq], axis=1))
        _CACHE["xdev"] = (jax.device_put(xt1, sharding),
                          jax.device_put(xr1, sharding))
        _CACHE["posfp"] = posfp
    if _CACHE.get("wfp") != wfp or "wdev" not in _CACHE:
        shared = _prep_weights_np(inputs)
        wdev = {}
        for name, arr in shared.items():
            rep = np.ascontiguousarray(
                np.broadcast_to(arr, (n_cores,) + arr.shape).reshape(
                    n_cores * arr.shape[0], *arr.shape[1:]))
            wdev[name] = jax.device_put(rep, sharding)
        _CACHE["wfp"] = wfp
        _CACHE["wdev"] = wdev


def _build_args(in_names):
    xt1_d, xr1_d = _CACHE["xdev"]
    wdev = _CACHE["wdev"]
    return [xt1_d if n == "xt1" else xr1_d if n == "xr1" else wdev[n]
            for n in in_names]


def _prev_bufs(zero_outs, sharding):
    # donated output buffers: reuse the previous call's outputs (the
    # kernel writes every element, so contents are irrelevant)
    import jax
    prev = _CACHE.pop("prev_out", None)
    if prev is None:
        prev = [
            jax.device_put(
                np.zeros((8 * z.shape[0], *z.shape[1:]), z.dtype),
                sharding)
            for z in zero_outs
        ]
    return prev


def _fetch(out_arrs):
    if ALLGATHER:
        # every shard is a full [B*P, 7] copy; fetch just one
        out = np.asarray(out_arrs[0].addressable_shards[0].data)
    else:
        out = np.asarray(out_arrs[0])  # [B*P, 7], cores in graph order
    _CACHE["prev_out"] = list(out_arrs)
    return out.astype(np.float32, copy=False), _Res()


def kernel(**inputs):
    out, _ = run_sharded(inputs)
    return out

